# revision 1
# baseline (speedup 1.0000x reference)
"""Trainium2 Bass kernel for nn_AdaptiveMixedCoding (8 NeuronCores).

Sharding: data-parallel over B_img (8 images per core); caps/cap_lens/alpha
replicated. Caption Grams computed split across cores and AllGathered.

Caption length-grouping: the NA=32 shortest captions (len <= WA=32) are
stored in 32-wide word slots ("A", 4 caps per 128-col unit at 32-offsets);
the rest keep 50-wide slots ("B", 2 caps per unit, packed 100 in the
similarity matrix). This shrinks the working width CWP from 3200 to
NA*32 + NB*50 = 2624, cutting S-matmul streaming, every elementwise/reduce
pass, and the qf unit count proportionally.

Per-core algorithm (Bi=8 imgs, R=36 regions, Bc=64 caps):
  S[row, cw] = dot(imgs[row], caps'[cw]) + adds  (fp16 matmul, K=1 ones row
               adds 0 valid / -30000 masked/pad into the same PSUM group)
  t          = S_sb * bc_scale'   (fp16; masked t ~= -300)
  exp        = Exp(t*invni10 - rowmax_all*invni10)  per-ROW max as scalar
               bias (per-caption max cancels in the softmax)
  hard       = (t == rowmax_c)  per-caption fp16 max, exact compare
  mixed'     = hard + exp * a/((1-a) den)
  num'       = sum_w mixed' * S_sb
  qf'        = mixed'^T G mixed'  (per-unit transposes -> M_T, u = M_T^T Gp,
               4 units per PSUM bank, strided products, 2 reduces)
  out        = num'/(sqrt(qf') + eps/(1-a)), invalid img rows -> -1

End-to-end l2 rel err vs the f32 reference ~8e-3 (fp16 t argmax ties).
"""
import sys
import contextlib

sys.path.insert(0, '/opt/trn_rl_repo')

import numpy as np

from concourse import bacc, tile, mybir

F32 = mybir.dt.float32
F16 = mybir.dt.float16
AF = mybir.ActivationFunctionType
OP = mybir.AluOpType
AX = mybir.AxisListType

N_CORES = 8
B, R, W, D = 64, 36, 50, 1024
BC = B
BI = B // N_CORES
ROWS = BI * R               # 288
KC = D // 128               # 8 contraction chunks
WA = 32                     # A-group stored words per caption (even)
ROW_TILES = [(0, 128), (128, 128), (256, 32)]
EPS = 1e-8
NEGS = -30000.0             # masked S offset; fp16-safe, *0.01 -> t ~= -300
KMASK = 0.01
TINY = 1e-30

_CACHE = {}


def _params(na):
    nb = BC - na
    cwa = na * WA            # A region width
    cwp = cwa + nb * W       # total packed width
    nu = na // 4 + nb // 2   # 128-col units (A: 4 caps, B: 2 caps)
    nupad = -(-nu // N_CORES) * N_CORES  # dummy units pad the AllGather
    upc = nupad // N_CORES   # gram units per core
    chunks = [(i * 512, min(512, cwp - i * 512))
              for i in range((cwp + 511) // 512)]
    return nb, cwa, cwp, nu, nupad, upc, chunks


def _build(a, na):
    am = max(a, 1e-6)
    oma = max(1.0 - a, 1e-6)
    NB, CWA, CWP, NU, NUPAD, UPC, N_CHUNKS = _params(na)
    NUA = na // 4            # A units
    GW = UPC * 128

    nc = bacc.Bacc("TRN2", target_bir_lowering=False, debug=False,
                   num_devices=N_CORES)

    capsT = nc.declare_dram_parameter("capsT", [D, CWP], F16, isOutput=False)
    gcaps = nc.declare_dram_parameter("gcaps", [D, GW], F16, isOutput=False)
    gcapso = nc.declare_dram_parameter("gcapso", [D, GW], F16,
                                       isOutput=False)  # 64-offset cols
    gmask = nc.declare_dram_parameter("gmask", [128, GW], F16, isOutput=False)
    imgsT = nc.declare_dram_parameter("imgsT", [D, ROWS], F16, isOutput=False)
    invni10_in = nc.declare_dram_parameter("invni10_in", [ROWS, 1], F32,
                                           isOutput=False)  # 10/||img row||
    bc_scale_in = nc.declare_dram_parameter("bc_scale_in", [128, CWP], F16,
                                            isOutput=False)  # invnc / KMASK
    addt_in = nc.declare_dram_parameter("addt_in", [128, CWP], F16,
                                         isOutput=False)  # 0 / -300 (t units)
    iv_col = nc.declare_dram_parameter("iv_col", [ROWS, 1], F32,
                                       isOutput=False)
    ivm1_col = nc.declare_dram_parameter("ivm1_col", [ROWS, 1], F32,
                                         isOutput=False)
    out_ext = nc.declare_dram_parameter("out", [BI, BC, R], F32, isOutput=True)

    gb_in = nc.dram_tensor("gb_in", [UPC, 128, 128], F16)
    gb_out = nc.dram_tensor("gb_out", [NUPAD, 128, 128], F16,
                            addr_space="Shared")

    with tile.TileContext(nc) as tc, contextlib.ExitStack() as ctx:
        const = ctx.enter_context(tc.tile_pool(name="const", bufs=1))
        big = ctx.enter_context(tc.tile_pool(name="big", bufs=1))
        work = ctx.enter_context(tc.tile_pool(name="work", bufs=2))
        work3 = ctx.enter_context(tc.tile_pool(name="work3", bufs=3))
        scr = ctx.enter_context(tc.tile_pool(name="scr", bufs=1))
        small = ctx.enter_context(tc.tile_pool(name="small", bufs=2))
        psS = ctx.enter_context(tc.tile_pool(name="psS", bufs=2, space="PSUM"))
        psQ = ctx.enter_context(tc.tile_pool(name="psQ", bufs=3, space="PSUM"))
        psM = ctx.enter_context(tc.tile_pool(name="psM", bufs=1, space="PSUM"))
        psT = ctx.enter_context(tc.tile_pool(name="psT", bufs=2, space="PSUM"))

        # ---- input loads: gcaps on the gpsimd DMA queue (parallel with
        # the sync queue carrying imgsT/caps) so grams start early ---------
        gcaps_sb = big.tile([128, KC, GW], F16)
        gcaps_r = gcaps.rearrange("(k p) m -> p k m", p=128)
        gcapso_sb = big.tile([128, KC, GW], F16)
        gcapso_r = gcapso.rearrange("(k p) m -> p k m", p=128)
        for j in range(UPC):
            nc.gpsimd.dma_start(out=gcaps_sb[:, :, 128 * j:128 * j + 128],
                                in_=gcaps_r[:, :, 128 * j:128 * j + 128])
            nc.gpsimd.dma_start(out=gcapso_sb[:, :, 128 * j:128 * j + 128],
                                in_=gcapso_r[:, :, 128 * j:128 * j + 128])
        imgsT_sb = big.tile([128, KC, ROWS], F16)
        nc.sync.dma_start(out=imgsT_sb[:],
                          in_=imgsT.rearrange("(k p) m -> p k m", p=128))
        caps_sb = big.tile([128, KC, CWP], F16)
        capsT_r = capsT.rearrange("(k p) m -> p k m", p=128)
        for (n0, nw) in N_CHUNKS:
            nc.sync.dma_start(out=caps_sb[:, :, n0:n0 + nw],
                              in_=capsT_r[:, :, n0:n0 + nw])
        bc_scale = big.tile([128, CWP], F16)
        nc.sync.dma_start(out=bc_scale[:], in_=bc_scale_in[:])

        addt_bc = big.tile([128, CWP], F16)
        nc.sync.dma_start(out=addt_bc[:], in_=addt_in[:])
        gmask_sb = const.tile([128, GW], F16)
        nc.gpsimd.dma_start(out=gmask_sb[:], in_=gmask[:])

        # constants built after the DMA issues (identity construction sits
        # on the gpsimd queue and would delay the gcaps dispatch)
        ident_16 = const.tile([128, 128], F16)
        from concourse.masks import make_identity
        make_identity(nc, ident_16[:])
        ident_f32 = const.tile([128, 128], F32)
        make_identity(nc, ident_f32[:])

        # ---- Grams for this core's UPC units (issued after S(t0) so the
        # PE head never blocks on the late gcaps DMA) ----------------------
        # Gloc[:, j, :] = (gcaps_j^T gcaps_j) * gmask_j  (cross blocks -> 0)
        Gloc = big.tile([128, UPC, 128], F16)
        Gp = big.tile([128, NUPAD, 128], F16)

        def gram_phase():
          for j in range(UPC):
              c0 = j * 128
              gps = psM.tile([128, 128], F32, tag="ps")
              for kc in range(KC):
                  nc.tensor.matmul(gps[:, :],
                                   gcaps_sb[:, kc, c0:c0 + 128],
                                   gcapso_sb[:, kc, c0:c0 + 128],
                                   start=(kc == 0), stop=(kc == KC - 1))
              nc.vector.tensor_tensor(Gloc[:, j, :], gps[:, :],
                                      gmask_sb[:, c0:c0 + 128], OP.mult)

          # gather all units (overlaps the S matmuls; needed only by qf)
          nc.gpsimd.dma_start(
              out=gb_in.rearrange("j r b -> r j b"),
              in_=Gloc[:])
          nc.gpsimd.collective_compute(
              "AllGather", OP.bypass,
              replica_groups=[list(range(N_CORES))],
              ins=[gb_in[:].opt()],
              outs=[gb_out[:].opt()],
          )
          nc.sync.dma_start(
              out=Gp[:, 0:NUPAD, :],
              in_=gb_out[:, :, :].rearrange("j r b -> r j b"))

        # transposed mixed, unit-block layout (built per row tile)
        M_T = big.tile([128, NU, ROWS], F16)
        nc.vector.memset(M_T[:, NU - 1, :], 0.0)

        # persistent output accumulator [BC, ROWS]
        out_sb = big.tile([BC, ROWS], F32)

        # unit table: (mixed col start, transpose width)
        units = [(4 * WA * u, min(128, CWP - 4 * WA * u))
                 for u in range(NUA)]
        units += [(CWA + 100 * v, min(128, CWP - CWA - 100 * v))
                  for v in range(NU - NUA)]

        # ---- pipelined per-row-tile phases ------------------------------
        def s_phase(r0, rt):
            mm = 128 if (r0 + 128 <= ROWS) else rt
            invni10 = small.tile([128, 1], F32, tag="invni10")
            nc.gpsimd.dma_start(out=invni10[:rt, :],
                                in_=invni10_in[r0:r0 + rt, :])
            iv_t = small.tile([128, 1], F32, tag="ivt")
            nc.gpsimd.dma_start(out=iv_t[:rt, :], in_=iv_col[r0:r0 + rt, :])
            ivm1_t = small.tile([128, 1], F32, tag="ivm1t")
            nc.gpsimd.dma_start(out=ivm1_t[:rt, :],
                                in_=ivm1_col[r0:r0 + rt, :])

            t = work3.tile([128, CWP], F16, tag="t")
            S_sb = work3.tile([128, CWP], F16, tag="S_sb")
            for (n0, nw) in N_CHUNKS:
                sps = psS.tile([128, 512], F32, tag="sps")
                for kc in range(KC):
                    nc.tensor.matmul(sps[:mm, :nw],
                                     imgsT_sb[:, kc, r0:r0 + mm],
                                     caps_sb[:, kc, n0:n0 + nw],
                                     start=(kc == 0), stop=(kc == KC - 1))
                nc.scalar.activation(S_sb[:rt, n0:n0 + nw], sps[:rt, :nw],
                                     AF.Copy)
                nc.vector.tensor_tensor(t[:rt, n0:n0 + nw],
                                        S_sb[:rt, n0:n0 + nw],
                                        bc_scale[:rt, n0:n0 + nw], OP.mult)
            nc.vector.tensor_tensor(t[:rt, :], t[:rt, :], addt_bc[:rt, :],
                                    OP.add)
            return t, S_sb, invni10, iv_t, ivm1_t

        def _red2(dst, x, rt, op):
            """dst[rt, BC] = per-caption reduce of x [rt, CWP] via a 2x-mode
            halving TT (pairs words w and w+h) then a half-width reduce."""
            half = scr.tile([128, CWP // 2], F16, tag="half")
            xa3 = (x[:rt, 0:CWA].rearrange("p (c w) -> p c w", w=WA)
                   if na else None)
            xb3 = x[:rt, CWA:CWP].rearrange("p (c w) -> p c w", w=W)
            hw_a, hw_b = WA // 2, W // 2
            ha = (half[:rt, 0:CWA // 2].rearrange("p (c w) -> p c w", w=hw_a)
                  if na else None)
            hb = half[:rt, CWA // 2:CWP // 2].rearrange(
                "p (c w) -> p c w", w=hw_b)
            if na:
                nc.vector.tensor_tensor(ha, xa3[:, :, 0:hw_a],
                                        xa3[:, :, hw_a:WA], op)
                nc.vector.tensor_reduce(dst[:rt, 0:na], ha,
                                        axis=AX.X, op=op)
            nc.vector.tensor_tensor(hb, xb3[:, :, 0:hw_b],
                                    xb3[:, :, hw_b:W], op)
            nc.vector.tensor_reduce(dst[:rt, na:BC], hb,
                                    axis=AX.X, op=op)

        def _ab(x, rt):
            """split [rt, CWP] into per-caption 3d views (A, B)."""
            xa = (x[:rt, 0:CWA].rearrange("p (c w) -> p c w", w=WA)
                  if na else None)
            xb = x[:rt, CWA:CWP].rearrange("p (c w) -> p c w", w=W)
            return xa, xb

        def v_pre(r0, rt, t, invni10):
            rowmax = small.tile([128, BC], F16, tag="rowmax")
            _red2(rowmax, t, rt, OP.max)
            nrm_all = small.tile([128, 1], F32, tag="nrmall")
            nc.vector.tensor_reduce(nrm_all[:rt, :], rowmax[:rt, :],
                                    axis=AX.X, op=OP.max, negate=True)
            nbias = small.tile([128, 1], F32, tag="nbias")
            nc.vector.tensor_scalar(nbias[:rt, :], nrm_all[:rt, :],
                                    invni10[:rt, :], None, OP.mult)
            el = work.tile([128, CWP], F16, tag="el")
            nc.scalar.activation(el[:rt, :], t[:rt, :], AF.Exp,
                                 bias=nbias[:rt, :], scale=invni10[:rt, :])
            bcbuf = work.tile([128, CWP], F16, tag="bcb")
            ba, bb = _ab(bcbuf, rt)
            if na:
                nc.scalar.activation(
                    ba, rowmax[:rt, 0:na, None].to_broadcast([rt, na, WA]),
                    AF.Copy)
            nc.scalar.activation(
                bb, rowmax[:rt, na:BC, None].to_broadcast([rt, NB, W]),
                AF.Copy)
            den = work3.tile([128, BC], F32, tag="den")
            _red2(den, el, rt, OP.add)
            invden = work3.tile([128, BC], F32, tag="invden")
            nc.vector.tensor_scalar(invden[:rt, :], den[:rt, :], oma / am,
                                    oma * TINY / am, OP.mult, OP.add)
            nc.vector.reciprocal(invden[:rt, :], invden[:rt, :])
            return el, bcbuf, invden

        def v_post(r0, rt, t, S_sb, el, bcbuf, invden):
            soft = work.tile([128, CWP], F16, tag="soft")
            sa, sb_ = _ab(soft, rt)
            ela, elb = _ab(el, rt)
            if na:
                nc.vector.tensor_tensor(
                    sa, ela,
                    invden[:rt, 0:na, None].to_broadcast([rt, na, WA]),
                    OP.mult)
            nc.vector.tensor_tensor(
                sb_, elb,
                invden[:rt, na:BC, None].to_broadcast([rt, NB, W]),
                OP.mult)
            mixed = work.tile([128, CWP], F16, tag="mixed")
            # hard into el (dead after soft), then mixed = soft + hard
            nc.vector.tensor_tensor(el[:rt, :], t[:rt, :], bcbuf[:rt, :],
                                    OP.is_equal)
            nc.vector.tensor_tensor(mixed[:rt, :], soft[:rt, :], el[:rt, :],
                                    OP.add)

            # num' = sum_w mixed * S  (prod into soft, dead now)
            nc.vector.tensor_tensor(soft[:rt, :], mixed[:rt, :], S_sb[:rt, :],
                                    OP.mult)
            num = small.tile([128, BC], F32, tag="num")
            _red2(num, soft, rt, OP.add)
            return mixed, num

        def qf_pe(r0, rt, mixed):
            """transposes + ups matmuls (PE), up to 4 units per PSUM bank."""
            for g0 in range(0, NU, 4):
                gl = min(4, NU - g0)
                tps = psT.tile([128, 512], F16, tag="tps")
                tws = []
                for pi in range(gl):
                    c0, tw = units[g0 + pi]
                    tws.append(tw)
                    nc.tensor.transpose(tps[0:tw, 128 * pi:128 * pi + rt],
                                        mixed[:rt, c0:c0 + tw],
                                        ident_16[0:rt, 0:rt])
                t4 = tps[:, :].rearrange("p (j x) -> p j x", x=128)
                if gl == 4 and min(tws) == 128:
                    nc.scalar.activation(
                        M_T[:, g0:g0 + 4, r0:r0 + rt],
                        t4[:, :, 0:rt], AF.Copy)
                else:
                    for pi in range(gl):
                        nc.scalar.activation(
                            M_T[0:tws[pi], g0 + pi, r0:r0 + rt],
                            t4[0:tws[pi], pi, 0:rt], AF.Copy)
            u_sb = scr.tile([128, NU * 128], F16, tag="usb")
            for g0 in range(0, NU, 4):
                gl = min(4, NU - g0)
                ups = psQ.tile([128, 512], F32, tag="ups")
                for pi in range(gl):
                    u = g0 + pi
                    nc.tensor.matmul(ups[:rt, 128 * pi:128 * pi + 128],
                                     M_T[:, u, r0:r0 + rt],
                                     Gp[:, u, :], start=True, stop=True)
                nc.scalar.activation(
                    u_sb[:rt, 128 * g0:128 * (g0 + gl)],
                    ups[:rt, 0:128 * gl], AF.Copy)
            return u_sb

        def qf_fin(r0, rt, mixed, u_sb, num, iv_t, ivm1_t):
            """qf products (2x fp16 SBUF), reduces, out row assembly."""
            qprod = work.tile([128, CWP], F16, tag="el")  # el ring reuse
            # one TT per region: unit slots 128 in u_sb, 4*WA (A) / 2*50 (B)
            # packed in mixed; strided 3d APs span all units at once
            if na:
                uA = u_sb[:rt, 0:128 * NUA].rearrange(
                    "p (j c) -> p j c", c=128)
                nc.vector.tensor_tensor(
                    qprod[:rt, 0:CWA].rearrange(
                        "p (j w) -> p j w", w=4 * WA),
                    mixed[:rt, 0:CWA].rearrange(
                        "p (j w) -> p j w", w=4 * WA),
                    uA[:, :, 0:4 * WA], OP.mult)
            uB = u_sb[:rt, 128 * NUA:128 * NU].rearrange(
                "p (j c) -> p j c", c=64)
            nc.vector.tensor_tensor(
                qprod[:rt, CWA:CWP].rearrange("p (j w) -> p j w", w=W),
                mixed[:rt, CWA:CWP].rearrange("p (j w) -> p j w", w=W),
                uB[:, :, 0:W], OP.mult)
            qf = small.tile([128, BC], F32, tag="qf")
            _red2(qf, qprod, rt, OP.add)

            denom = small.tile([128, BC], F32, tag="denom")
            nc.scalar.activation(denom[:rt, :], qf[:rt, :], AF.Sqrt)
            nc.vector.tensor_scalar(denom[:rt, :], denom[:rt, :], EPS / oma,
                                    None, OP.add)
            nc.vector.reciprocal(denom[:rt, :], denom[:rt, :])
            res = small.tile([128, BC], F32, tag="res")
            nc.vector.tensor_tensor(res[:rt, :], num[:rt, :], denom[:rt, :],
                                    OP.mult)
            nc.vector.tensor_scalar(res[:rt, :], res[:rt, :], iv_t[:rt, :],
                                    ivm1_t[:rt, :], OP.mult, OP.add)

            ops_ = psM.tile([BC, 128], F32, tag="ps")
            nc.tensor.transpose(ops_[:, :rt], res[:rt, :],
                                ident_f32[0:rt, 0:rt])
            nc.scalar.activation(out_sb[:, r0:r0 + rt], ops_[:, :rt], AF.Copy)
            i0 = r0 // R                  # first image not yet flushed
            i1 = (r0 + rt) // R           # images complete after this tile
            if i1 > i0:
                nc.scalar.dma_start(
                    out=out_ext[i0:i1].rearrange("i c r -> c i r"),
                    in_=out_sb[:, i0 * R:i1 * R].rearrange(
                        "c (i r) -> c i r", r=R))

        # grams + collective, all S phases, then all pre-exp chains
        # (fills vector/scalar bubbles across tiles), then post/qf staggered
        gram_phase()
        st = [s_phase(*ROW_TILES[i]) for i in range(3)]
        pres = [v_pre(ROW_TILES[i][0], ROW_TILES[i][1], st[i][0], st[i][2])
                for i in range(3)]
        pend = None
        for i in range(3):
            r0, rt = ROW_TILES[i]
            mi, ni = v_post(r0, rt, st[i][0], st[i][1], *pres[i])
            if pend is not None:
                qf_fin(*pend)
            usb = qf_pe(r0, rt, mi)
            pend = (r0, rt, mi, usb, ni, st[i][3], st[i][4])
        qf_fin(*pend)


    nc.finalize()
    return nc


def _get_runner(a, na):
    key = (round(float(a), 9), na)
    if key not in _CACHE:
        _CACHE[key] = _build(*key)
    return _CACHE[key]


def _host_prep(imgs, caps, img_lens, cap_lens, na, perm):
    NB, CWA, CWP, NU, NUPAD, UPC, _ = _params(na)
    NUA = na // 4
    imgs = np.ascontiguousarray(np.asarray(imgs, dtype=np.float32))
    caps = np.ascontiguousarray(np.asarray(caps, dtype=np.float32))
    img_lens = np.asarray(img_lens).astype(np.int64)
    cap_lens = np.asarray(cap_lens).astype(np.int64)

    capsTf = np.ascontiguousarray(
        caps.reshape(BC * W, D).T).reshape(D, BC, W)  # [D, c, w] f32
    # packed layout: A caps (perm[:na]) in 32-wide slots, B in 50-wide
    capsT = np.zeros((D, CWP), dtype=np.float16)
    valid = np.zeros(CWP, dtype=np.float32)
    inv_nc = 1.0 / (np.linalg.norm(caps.astype(np.float64), axis=-1) + EPS)
    scale = np.full(CWP, KMASK, dtype=np.float32)
    for j, c in enumerate(perm):
        if j < na:
            sl = slice(WA * j, WA * j + WA)
            ww = WA
        else:
            sl = slice(CWA + W * (j - na), CWA + W * (j - na) + W)
            ww = W
        capsT[:, sl] = capsTf[:, c, 0:ww].astype(np.float16)
        v = (np.arange(ww) < cap_lens[c]).astype(np.float32)
        valid[sl] = v
        scale[sl] = np.where(v > 0, inv_nc[c, 0:ww], KMASK)

    addt = np.where(valid > 0, 0.0, NEGS * KMASK).astype(np.float16)
    addt_in = np.ascontiguousarray(
        np.broadcast_to(addt[None, :], (128, CWP)))
    bc_scale_in = np.ascontiguousarray(
        np.broadcast_to(scale.astype(np.float16)[None, :], (128, CWP)))

    # gram inputs per core: UPC units of 128 cols
    # A unit u: caps perm[4u..4u+4] at 32-offsets == capsT slice
    # B unit v: caps perm[na+2v], perm[na+2v+1] at 0:50 / 64:114
    gmask_u = np.zeros((NUPAD, 128, 128), dtype=np.float16)
    gcaps_u = np.zeros((NUPAD, D, 128), dtype=np.float16)
    gcapso_u = np.zeros((NUPAD, D, 128), dtype=np.float16)
    uw = 4 * WA
    for u in range(NUA):
        gcaps_u[u, :, 0:uw] = capsT[:, uw * u:uw * u + uw]
        gcapso_u[u, :, 0:uw] = capsT[:, uw * u:uw * u + uw]
        for aa in range(4):
            gmask_u[u, WA * aa:WA * aa + WA, WA * aa:WA * aa + WA] = 1.0
    for v in range(NU - NUA):
        u = NUA + v
        c0, c1 = perm[na + 2 * v], perm[na + 2 * v + 1]
        gcaps_u[u, :, 0:50] = capsTf[:, c0, :].astype(np.float16)
        gcaps_u[u, :, 50:100] = capsTf[:, c1, :].astype(np.float16)
        gcapso_u[u, :, 0:50] = capsTf[:, c0, :].astype(np.float16)
        gcapso_u[u, :, 64:114] = capsTf[:, c1, :].astype(np.float16)
        gmask_u[u, 0:50, 0:50] = 1.0
        gmask_u[u, 50:100, 64:114] = 1.0

    in_maps = []
    for core in range(N_CORES):
        sl = slice(core * BI, (core + 1) * BI)
        im = imgs[sl].reshape(ROWS, D)
        imT = np.ascontiguousarray(im.T).astype(np.float16)
        invni10 = (10.0 / (np.linalg.norm(im.astype(np.float64), axis=1)
                           + EPS)).astype(np.float32).reshape(ROWS, 1)
        iv = (np.arange(R)[None, :] < img_lens[sl][:, None]).astype(
            np.float32).reshape(ROWS, 1)
        usl = slice(core * UPC, (core + 1) * UPC)
        in_maps.append({
            "capsT": capsT,
            "gcaps": np.ascontiguousarray(
                gcaps_u[usl].transpose(1, 0, 2).reshape(D, UPC * 128)),
            "gcapso": np.ascontiguousarray(
                gcapso_u[usl].transpose(1, 0, 2).reshape(D, UPC * 128)),
            "gmask": np.ascontiguousarray(
                gmask_u[usl].transpose(1, 0, 2).reshape(128, UPC * 128)),
            "imgsT": imT,
            "invni10_in": invni10,
            "bc_scale_in": bc_scale_in,
            "addt_in": addt_in,
            "iv_col": iv,
            "ivm1_col": iv - 1.0,
        })
    return in_maps


def run_on_device(inputs: dict, trace: bool = False):
    """Returns (output [64,64,36] f32, BassKernelResults)."""
    from concourse.bass_utils import run_bass_kernel_spmd
    alpha = float(np.asarray(inputs["alpha"]).reshape(-1)[0])
    a = 1.0 / (1.0 + np.exp(-alpha))
    cap_lens = np.asarray(inputs["cap_lens"]).astype(np.int64)
    order = np.argsort(cap_lens, kind="stable")
    na = 4 * (int((cap_lens <= WA).sum()) // 4)
    perm = np.asarray(order if na else np.arange(BC))
    nc = _get_runner(a, na)
    in_maps = _host_prep(inputs["imgs"], inputs["caps"], inputs["img_lens"],
                         cap_lens, na, perm)
    r = run_bass_kernel_spmd(nc, in_maps, list(range(N_CORES)), trace=trace)
    dev = np.concatenate([r.results[c]["out"][None] for c in range(N_CORES)],
                         axis=0).reshape(B, BC, R).astype(np.float32)
    out = np.empty_like(dev)
    out[:, perm, :] = dev
    return out, r


def kernel(imgs, caps, img_lens, cap_lens, alpha):
    out, _ = run_on_device({"imgs": imgs, "caps": caps, "img_lens": img_lens,
                            "cap_lens": cap_lens, "alpha": alpha})
    return out

